# revision 8
# baseline (speedup 1.0000x reference)
"""AmpPerLoss distributed Trainium2 kernel (v3: engine-balanced pipeline).

Data-parallel over batch: 128 samples / 8 cores = 16 per core; each
sample's 100000-length row spans 8 SBUF partitions x 12500, so a core
shard is [128, 12500] (partition p = 8*sample + chunk).

Per core:
  - Loads ordered p -> t -> s in 2-block chunks; every consumer is
    chunk-pipelined behind its DMA.
  - BCE: sum softplus(p) = -sum ln(1 - sigmoid(p)) in ONE extra ACT pass
    (Ln with scale=-1, bias=1 reads the sigmoid tensor); sum p*t on the
    TensorEngine via 100 uniform 125-col accumulating matmuls + an
    identity-mask diagonal extract.
  - Window bounds: per-block maxes of t/p via 2x-packed halving folds
    (even inner dims keep DVE bf16 2x packing), tiny encode algebra,
    DRAM-bounce regroups, one fused gather (t+p+s pads concatenated in
    DRAM) and an exact in-block refine with per-row ramps/thresholds.
  - Windowed amplitude: interior from per-block min/max of s (masked
    block reduces), edges from gathered s blocks.
  - Smoothness: chunked shifted subtract of sigmoid (fp16, DVE 2x) +
    chunked ACT Abs accumulators; row-boundary pairs finished on host.

Host reduces the 8 cores' partial stats to the final scalar.
"""

import sys

if "/opt/trn_rl_repo" not in sys.path:
    sys.path.insert(0, "/opt/trn_rl_repo")

from contextlib import ExitStack

import numpy as np

import concourse.bass as bass
import concourse.bacc as bacc
import concourse.tile as tile
import concourse.mybir as mybir
from concourse.bass_utils import run_bass_kernel_spmd
from concourse import dve_ops

N_CORES = 8
B, L = 128, 100000
BPC = B // N_CORES          # samples per core
CHUNKS = 8                  # partitions per sample
P = BPC * CHUNKS            # 128 partitions
F = L // CHUNKS             # 12500 free elements per row
NB = 10                     # blocks per row
SUB = F // NB               # 1250
PB = 1280                   # padded block length in DRAM (256B aligned)
NCH = 5                     # load chunks per tensor (2 blocks each)
FCH = F // NCH              # 2500
H1, H2, H3, H4 = 626, 314, 158, 80   # even fold sizes
MMW = 125                   # matmul chunk width (100 uniform chunks)
NMM = F // MMW
BIGF = 1.0e30
FMIN = -3.0e38

F32 = mybir.dt.float32
BF16 = mybir.dt.bfloat16
FP16 = mybir.dt.float16
I16 = mybir.dt.int16
Alu = mybir.AluOpType
Act = mybir.ActivationFunctionType
AX = mybir.AxisListType

# stats column layout ([P, NSTAT])
C_WMAX_T, C_WMIN_T, C_WMAX_P, C_WMIN_P = 0, 1, 2, 3
C_SP, C_PT = 4, 5           # C_SP is first of 4 cols: 4, 13, 14, 15
C_SM0 = 6                   # 5 cols: 6..10
C_SIG0, C_SIGL = 11, 12
SP_COLS = (4, 13, 14, 15)
NSTAT = 16


def build_nc(n_cores=N_CORES):
    nc = bacc.Bacc("TRN2", target_bir_lowering=False, debug=False,
                   num_devices=n_cores)

    p_ext = nc.dram_tensor("p", [P, F], BF16, kind="ExternalInput")
    t_ext = nc.dram_tensor("t", [P, F], BF16, kind="ExternalInput")
    s_ext = nc.dram_tensor("s", [P, F], BF16, kind="ExternalInput")
    pad_ext = nc.dram_tensor("tps_pad", [3 * P * NB, PB], BF16,
                             kind="ExternalInput")
    ascB_ext = nc.dram_tensor("ascB", [P, NB], F32, kind="ExternalInput")
    descB_ext = nc.dram_tensor("descB", [P, NB], F32, kind="ExternalInput")
    korrB_ext = nc.dram_tensor("korrB", [P, 4], F32, kind="ExternalInput")
    sgnB_ext = nc.dram_tensor("sgnB", [P, 4], F32, kind="ExternalInput")
    bigsB_ext = nc.dram_tensor("bigsB", [P, 4], F32, kind="ExternalInput")
    offB_ext = nc.dram_tensor("offB", [P, 1], F32, kind="ExternalInput")
    i80_ext = nc.dram_tensor("i80", [BPC, 1], F32, kind="ExternalInput")
    goff_ext = nc.dram_tensor("goff", [BPC, 8], F32, kind="ExternalInput")
    thr_ext = nc.dram_tensor("thr", [64, 1], F32, kind="ExternalInput")
    rramp_ext = nc.dram_tensor("rramp", [64, PB], FP16, kind="ExternalInput")
    ident_ext = nc.dram_tensor("ident", [P, MMW], BF16, kind="ExternalInput")

    stats_ext = nc.dram_tensor("stats", [P, NSTAT], F32, kind="ExternalOutput")
    b16_ext = nc.dram_tensor("b16", [BPC, 4], F32, kind="ExternalOutput")
    edge_ext = nc.dram_tensor("edge", [64, 2], F32, kind="ExternalOutput")

    # DRAM bounce buffers (partition-crossing regroups)
    rv_b = nc.dram_tensor("rv_b", [P, 4], F32)
    pk_b = nc.dram_tensor("pk_b", [BPC, 4], F32)
    enc_b = nc.dram_tensor("enc_b", [64, 1], F32)
    sb_b = nc.dram_tensor("sb_b", [BPC, 8], F32)

    ctx = ExitStack()
    with tile.TileContext(nc) as tc, ctx:
        big = ctx.enter_context(tc.tile_pool(name="big", bufs=1))
        small = ctx.enter_context(tc.tile_pool(name="small", bufs=1))
        psum_pool = ctx.enter_context(
            tc.tile_pool(name="psum", bufs=1, space="PSUM"))

        p_sb = big.tile([P, F], BF16, tag="P")
        t_sb = big.tile([P, F], BF16, tag="T")
        s_sb = big.tile([P, F], BF16, tag="S")
        sig = big.tile([P, F], FP16, tag="SIG")

        # consts
        ascB = small.tile([P, NB], F32, tag="ascB")
        descB = small.tile([P, NB], F32, tag="descB")
        korrB = small.tile([P, 4], F32, tag="korrB")
        sgnB = small.tile([P, 4], F32, tag="sgnB")
        bigsB = small.tile([P, 4], F32, tag="bigsB")
        offB = small.tile([P, 1], F32, tag="offB")
        i80 = small.tile([BPC, 1], F32, tag="i80")
        goff = small.tile([BPC, 8], F32, tag="goff")
        thr = small.tile([64, 1], F32, tag="thr")
        rramp = small.tile([64, PB], FP16, tag="rramp")
        ident = small.tile([P, MMW], BF16, tag="ident")
        for sb, ext in ((ascB, ascB_ext), (descB, descB_ext),
                        (korrB, korrB_ext), (sgnB, sgnB_ext),
                        (bigsB, bigsB_ext), (offB, offB_ext),
                        (i80, i80_ext), (goff, goff_ext), (thr, thr_ext),
                        (rramp, rramp_ext), (ident, ident_ext)):
            nc.sync.dma_start(out=sb, in_=ext.ap())

        stats = small.tile([P, NSTAT], F32, tag="stats")
        nc.vector.memset(stats[:, :], 0.0)

        # ---- loads: p first, then t, then s (2-block chunks)
        for k in range(NCH):
            sl = slice(k * FCH, (k + 1) * FCH)
            nc.sync.dma_start(out=p_sb[:, sl], in_=p_ext.ap()[:, sl])
        for k in range(NCH):
            sl = slice(k * FCH, (k + 1) * FCH)
            nc.sync.dma_start(out=t_sb[:, sl], in_=t_ext.ap()[:, sl])
        for k in range(NCH):
            sl = slice(k * FCH, (k + 1) * FCH)
            nc.sync.dma_start(out=s_sb[:, sl], in_=s_ext.ap()[:, sl])

        # ---- ACT: sigmoid chunks follow p
        for k in range(NCH):
            sl = slice(k * FCH, (k + 1) * FCH)
            nc.scalar.activation(out=sig[:, sl], in_=p_sb[:, sl],
                                 func=Act.Sigmoid)
        # ACT: sum ln(1 - sig) = -sum softplus(p), 4 chunks of 3125
        lndump = big.tile([P, F // 4], BF16, tag="LND")
        for k in range(4):
            sl = slice(k * (F // 4), (k + 1) * (F // 4))
            nc.scalar.activation(out=lndump[:, :], in_=sig[:, sl],
                                 func=Act.Ln, scale=-1.0, bias=1.0,
                                 accum_out=stats[:, SP_COLS[k]:SP_COLS[k] + 1])

        # ---- fold chains (DVE 2x): per-chunk fold1, then fold2-4 + reduce
        f1a = big.tile([P, NB, H1], BF16, tag="F1A")   # p, then s-max
        f1b = big.tile([P, NB, H1], BF16, tag="F1B")   # t, then s-min
        g2 = big.tile([P, NB, H2], BF16, tag="G2")
        g3 = big.tile([P, NB, H3], BF16, tag="G3")
        g4 = big.tile([P, NB, H4], BF16, tag="G4")
        bmax_p = small.tile([P, NB], F32, tag="bmax_p")
        bmax_t = small.tile([P, NB], F32, tag="bmax_t")
        bmax_s = small.tile([P, NB], F32, tag="bmax_s")
        bmin_s = small.tile([P, NB], F32, tag="bmin_s")

        def fold1(dst, src, k, op):
            v = src[:, k * FCH:(k + 1) * FCH].rearrange(
                "q (b f) -> q b f", b=2)
            nc.vector.tensor_tensor(out=dst[:, 2 * k:2 * k + 2, :],
                                    in0=v[:, :, 0:H1],
                                    in1=v[:, :, SUB - H1:SUB], op=op)

        def chain_tail(f1, out, op):
            nc.vector.tensor_tensor(out=g2[:, :, :], in0=f1[:, :, 0:H2],
                                    in1=f1[:, :, H1 - H2:H1], op=op)
            nc.vector.tensor_tensor(out=g3[:, :, :], in0=g2[:, :, 0:H3],
                                    in1=g2[:, :, H2 - H3:H2], op=op)
            nc.vector.tensor_tensor(out=g4[:, :, :], in0=g3[:, :, 0:H4],
                                    in1=g3[:, :, H3 - H4:H3], op=op)
            nc.vector.tensor_reduce(out=out[:, :], in_=g4[:, :, :],
                                    axis=AX.X, op=op)

        # smoothness sub chunks (fp16 2x); separate d tiles so ACT lag
        # never back-pressures DVE
        dch = []
        for k in range(NCH):
            d_k = big.tile([P, FCH], FP16, tag=f"D{k}", name=f"d{k}")
            dch.append(d_k)

        def sub_chunk(k):
            a = k * FCH
            w = FCH if k < NCH - 1 else FCH - 1
            nc.vector.tensor_sub(dch[k][:, 0:w], sig[:, a + 1:a + 1 + w],
                                 sig[:, a:a + w])

        # DVE: p-chain during p load
        for k in range(NCH):
            fold1(f1a, p_sb, k, Alu.max)
        chain_tail(f1a, bmax_p, Alu.max)
        # DVE: t-chain during t load, subs 0-2 filling arrival gaps
        fold1(f1b, t_sb, 0, Alu.max)
        fold1(f1b, t_sb, 1, Alu.max)
        sub_chunk(0)
        fold1(f1b, t_sb, 2, Alu.max)
        sub_chunk(1)
        fold1(f1b, t_sb, 3, Alu.max)
        sub_chunk(2)
        fold1(f1b, t_sb, 4, Alu.max)
        chain_tail(f1b, bmax_t, Alu.max)

        # ---- matmuls: p^T @ t in 100 uniform 125-col chunks (PE queue)
        psum = psum_pool.tile([P, MMW], F32)
        for c in range(NMM):
            nc.tensor.matmul(out=psum[0:MMW, 0:MMW],
                             lhsT=p_sb[:, c * MMW:(c + 1) * MMW],
                             rhs=t_sb[:, c * MMW:(c + 1) * MMW],
                             start=(c == 0), stop=(c == NMM - 1))

        # ---- block-level bound encode (tiny)
        anyt = small.tile([P, NB], F32, tag="anyt")
        anyp = small.tile([P, NB], F32, tag="anyp")
        nc.vector.tensor_scalar(out=anyt[:, :], in0=bmax_t[:, :],
                                scalar1=0.5, scalar2=None, op0=Alu.is_gt)
        nc.vector.tensor_scalar(out=anyp[:, :], in0=bmax_p[:, :],
                                scalar1=0.0, scalar2=None, op0=Alu.is_gt)
        encB = small.tile([P, 4], F32, tag="encB")  # [hi_t, hi_p, lo_t, lo_p]
        ze = small.tile([P, NB], F32, tag="ze")
        for i, (src, rmp) in enumerate(((anyt, ascB), (anyp, ascB),
                                        (anyt, descB), (anyp, descB))):
            nc.vector.tensor_mul(ze[:, :], src[:, :], rmp[:, :])
            nc.vector.tensor_reduce(out=encB[:, i:i + 1], in_=ze[:, :],
                                    axis=AX.X, op=Alu.max)
        # hi cols: g = enc-1 + 10c (else -BIG); lo: g = 10-enc + 10c (else BIG)
        cm = small.tile([P, 4], F32, tag="cm")
        dm = small.tile([P, 4], F32, tag="dm")
        a1 = small.tile([P, 4], F32, tag="a1")
        t1 = small.tile([P, 4], F32, tag="t1")
        t2 = small.tile([P, 4], F32, tag="t2")
        rowvals = small.tile([P, 4], F32, tag="rowvals")
        nc.vector.tensor_scalar(out=cm[:, :], in0=encB[:, :], scalar1=0.0,
                                scalar2=None, op0=Alu.is_gt)
        nc.vector.tensor_scalar(out=dm[:, :], in0=encB[:, :], scalar1=0.0,
                                scalar2=None, op0=Alu.is_le)
        nc.vector.tensor_mul(a1[:, :], encB[:, :], sgnB[:, :])
        nc.vector.tensor_add(a1[:, :], a1[:, :], korrB[:, :])
        nc.vector.tensor_scalar(out=a1[:, :], in0=a1[:, :],
                                scalar1=offB[:, 0:1], scalar2=None,
                                op0=Alu.add)
        nc.vector.tensor_mul(t1[:, :], cm[:, :], a1[:, :])
        nc.vector.tensor_mul(t2[:, :], dm[:, :], bigsB[:, :])
        nc.vector.tensor_add(rowvals[:, :], t1[:, :], t2[:, :])

        # ---- per-sample combine: bounce [P,4] -> [16,4,8], reduce over 8
        comb = small.tile([BPC, CHUNKS, 4], F32, tag="comb")
        nc.sync.dma_start(out=rv_b.ap(), in_=rowvals[:, :])
        rap = rv_b.ap()
        # comb[i, c, k] = rv_b[8i + c, k]
        nc.sync.dma_start(out=comb[:, :, :], in_=bass.AP(
            tensor=rap.tensor, offset=rap.offset,
            ap=[[4 * CHUNKS, BPC], [4, CHUNKS], [1, 4]]))
        sub_chunk(3)
        combv = comb[:, :, :].rearrange("b c k -> b k c")
        pack = small.tile([BPC, 4], F32, tag="pack")
        nc.vector.tensor_reduce(out=pack[:, 0:2], in_=combv[:, 0:2, :],
                                axis=AX.X, op=Alu.max)
        nc.vector.tensor_reduce(out=pack[:, 2:4], in_=combv[:, 2:4, :],
                                axis=AX.X, op=Alu.min)
        nc.sync.dma_start(out=b16_ext.ap(), in_=pack[:, :])

        # eq per mask: single-block window (lo_g == hi_g)
        eq2 = small.tile([BPC, 2], F32, tag="eq2")
        nc.vector.tensor_tensor(out=eq2[:, :], in0=pack[:, 2:4],
                                in1=pack[:, 0:2], op=Alu.is_equal)

        # ---- gather indices: idx8 = clamp(g,0,79) + 80*i + goff
        gcl = small.tile([BPC, 4], F32, tag="gcl")
        for dst_c, src_c in ((0, 2), (1, 0), (2, 3), (3, 1)):
            nc.vector.tensor_copy(gcl[:, dst_c:dst_c + 1],
                                  pack[:, src_c:src_c + 1])
        nc.vector.tensor_scalar(out=gcl[:, :], in0=gcl[:, :],
                                scalar1=0.0, scalar2=79.0,
                                op0=Alu.max, op1=Alu.min)
        nc.vector.tensor_scalar(out=gcl[:, :], in0=gcl[:, :],
                                scalar1=i80[:, 0:1], scalar2=None,
                                op0=Alu.add)
        idx8 = small.tile([BPC, 8], F32, tag="idx8")
        nc.vector.tensor_copy(idx8[:, 0:4], gcl[:, :])
        nc.vector.tensor_copy(idx8[:, 4:8], gcl[:, :])
        nc.vector.tensor_add(idx8[:, :], idx8[:, :], goff[:, :])
        idx_i = small.tile([BPC, 8], I16, tag="idx_i")
        nc.vector.tensor_copy(idx_i[:, :], idx8[:, :])

        # ---- broadcast per-sample bounds to rows (interior masks)
        rb = small.tile([P, 4], F32, tag="rb")
        nc.sync.dma_start(out=pk_b.ap(), in_=pack[:, :])
        pap = pk_b.ap()
        # rb[8i + c, :] = pk_b[i, :]
        nc.sync.dma_start(out=rb[:, :], in_=bass.AP(
            tensor=pap.tensor, offset=pap.offset,
            ap=[[4, BPC], [0, CHUNKS], [1, 4]]))

        # ---- gathers (fused pad tensor; manual DMA semaphores)
        gref = small.tile([P, 1, PB], BF16, tag="gref")
        sgat = small.tile([P, 1, PB], BF16, tag="sgat")
        from concourse.bass import _add_dep_helper
        gsem_r = nc.alloc_semaphore("gsem_r")
        gsem_s = nc.alloc_semaphore("gsem_s")
        nc.gpsimd.dma_gather(
            out_ap=gref[:, :, :], in_ap=pad_ext.ap(),
            idxs_ap=idx_i[:, 0:4], num_idxs=64,
            num_idxs_reg=64, elem_size=PB,
            prepare_only=True, sem=gsem_r)
        nc.gpsimd.dma_gather(
            out_ap=sgat[:, :, :], in_ap=pad_ext.ap(),
            idxs_ap=idx_i[:, 4:8], num_idxs=64,
            num_idxs_reg=64, elem_size=PB,
            prepare_only=True, sem=gsem_s)
        trig = nc.gpsimd.trigger_dma(count=None)
        gw = {}
        for key, sem in (("r", gsem_r), ("s", gsem_s)):
            w = nc.gpsimd.wait_ge(sem, 16)
            _add_dep_helper(w.ins, trig.ins, sync=False,
                            reason="gather wait after trigger")
            gw[key] = w

        def dep_on_gather(inst, key):
            _add_dep_helper(inst.ins, gw[key].ins, sync=True,
                            reason=f"reader waits {key}-gather")

        # ---- s fold chains (interleaved max/min) + refine parked mid-way
        sub_chunk(4)
        fold1(f1a, s_sb, 0, Alu.max)
        fold1(f1b, s_sb, 0, Alu.min)
        fold1(f1a, s_sb, 1, Alu.max)
        fold1(f1b, s_sb, 1, Alu.min)

        # refine exact in-block positions (rows: lo_t, hi_t, lo_p, hi_p);
        # these park in DVE wait slots until the gather semaphore fires
        refm = small.tile([64, PB], FP16, tag="refm")
        refe = small.tile([64, PB], FP16, tag="refe")
        enc = small.tile([64, 1], F32, tag="enc")
        r_ = nc.vector.tensor_scalar(out=refm[:, :], in0=gref[0:64, 0, :],
                                     scalar1=thr[:, 0:1], scalar2=None,
                                     op0=Alu.is_gt)
        dep_on_gather(r_, "r")
        nc.vector.tensor_mul(refe[:, :], refm[:, :], rramp[:, :])
        nc.vector.tensor_reduce(out=enc[:, :], in_=refe[:, :],
                                axis=AX.X, op=Alu.max)
        # regroup enc -> encs16[i, g]
        encs16 = small.tile([BPC, 4], F32, tag="encs16")
        nc.sync.dma_start(out=enc_b.ap(), in_=enc[:, :])
        eap = enc_b.ap()
        nc.sync.dma_start(out=encs16[:, :], in_=bass.AP(
            tensor=eap.tensor, offset=eap.offset,
            ap=[[1, BPC], [BPC, 4]]))

        fold1(f1a, s_sb, 2, Alu.max)
        fold1(f1b, s_sb, 2, Alu.min)

        # sgneg for the min-edge reduce (dep on s-gather)
        sgneg = small.tile([64, PB], BF16, tag="sgneg")
        r_ = nc.vector.tensor_scalar(out=sgneg[:, :], in0=sgat[0:64, 0, :],
                                     scalar1=-1.0, scalar2=None, op0=Alu.mult)
        dep_on_gather(r_, "s")

        fold1(f1a, s_sb, 3, Alu.max)
        fold1(f1b, s_sb, 3, Alu.min)
        fold1(f1a, s_sb, 4, Alu.max)
        fold1(f1b, s_sb, 4, Alu.min)

        # ---- per-row [start, end) for gathered s blocks
        # encs16 cols: [enc_lo_t, enc_hi_t, enc_lo_p, enc_hi_p]
        sb8 = small.tile([BPC, 8], F32, tag="sb8")
        tmp2 = small.tile([BPC, 2], F32, tag="tmp2")
        for c, ec in ((0, 0), (4, 2)):   # st_lo = 1250 - enc_lo
            nc.vector.tensor_scalar(
                out=sb8[:, c:c + 1], in0=encs16[:, ec:ec + 1],
                scalar1=-1.0, scalar2=float(SUB), op0=Alu.mult, op1=Alu.add)
        for c, ec in ((0, 1), (1, 3)):   # en_lo = 1250 + eq*(enc_hi - 1250)
            nc.vector.tensor_scalar(
                out=tmp2[:, c:c + 1], in0=encs16[:, ec:ec + 1],
                scalar1=-float(SUB), scalar2=None, op0=Alu.add)
        nc.vector.tensor_mul(tmp2[:, :], tmp2[:, :], eq2[:, :])
        for c in (0, 1):
            nc.vector.tensor_scalar(
                out=sb8[:, 4 * c + 1:4 * c + 2], in0=tmp2[:, c:c + 1],
                scalar1=float(SUB), scalar2=None, op0=Alu.add)
        for c in (0, 1):                 # st_hi = eq * st_lo
            nc.vector.tensor_mul(sb8[:, 4 * c + 2:4 * c + 3],
                                 eq2[:, c:c + 1], sb8[:, 4 * c:4 * c + 1])
        for c, ec in ((0, 1), (1, 3)):   # en_hi = enc_hi
            nc.vector.tensor_copy(sb8[:, 4 * c + 3:4 * c + 4],
                                  encs16[:, ec:ec + 1])
        sbnd = small.tile([64, 2], F32, tag="sbnd")
        nc.sync.dma_start(out=sb_b.ap(), in_=sb8[:, :])
        sap = sb_b.ap()
        nc.sync.dma_start(out=sbnd[:, :], in_=bass.AP(
            tensor=sap.tensor, offset=sap.offset,
            ap=[[2, 4], [8, BPC], [1, 2]]))

        # ---- edge extremes from gathered s blocks
        edge = small.tile([64, 2], F32, tag="edge")
        edump = small.tile([64, PB], BF16, tag="edump")
        r_ = nc.vector._custom_dve(
            dve_ops.TENSOR_MASK_REDUCE,
            out=edump[:, :], in0=sgat[0:64, 0, :], in1=sbnd[:, 1:2],
            s0=sbnd[:, 0:1], s1=FMIN, imm2=1.0, accum_out=edge[:, 0:1])
        dep_on_gather(r_, "s")
        nc.vector._custom_dve(
            dve_ops.TENSOR_MASK_REDUCE,
            out=edump[:, :], in0=sgneg[:, :], in1=sbnd[:, 1:2],
            s0=sbnd[:, 0:1], s1=FMIN, imm2=1.0, accum_out=edge[:, 1:2])
        nc.sync.dma_start(out=edge_ext.ap(), in_=edge[:, :])

        # ---- s chain tails
        chain_tail(f1a, bmax_s, Alu.max)
        chain_tail(f1b, bmin_s, Alu.min)

        # ---- interior extremes from block stats (masked block reduces)
        ibs = small.tile([P, 2], F32, tag="ibs")
        ibe = small.tile([P, 2], F32, tag="ibe")
        nc.vector.tensor_scalar(out=ibs[:, :], in0=rb[:, 2:4],
                                scalar1=offB[:, 0:1], scalar2=1.0,
                                op0=Alu.subtract, op1=Alu.add)
        nc.vector.tensor_scalar(out=ibe[:, :], in0=rb[:, 0:2],
                                scalar1=offB[:, 0:1], scalar2=None,
                                op0=Alu.subtract)
        nc.vector.tensor_tensor(out=ibs[:, :], in0=ibs[:, :], in1=ibe[:, :],
                                op=Alu.min)
        negb = small.tile([P, NB], F32, tag="negb")
        nc.vector.tensor_scalar(out=negb[:, :], in0=bmin_s[:, :],
                                scalar1=-1.0, scalar2=None, op0=Alu.mult)
        bdump = small.tile([P, NB], F32, tag="bdump")
        for (data, scol, ccol) in ((bmax_s, 0, C_WMAX_T), (negb, 0, C_WMIN_T),
                                   (bmax_s, 1, C_WMAX_P), (negb, 1, C_WMIN_P)):
            nc.vector._custom_dve(
                dve_ops.TENSOR_MASK_REDUCE,
                out=bdump[:, :], in0=data[:, :], in1=ibe[:, scol:scol + 1],
                s0=ibs[:, scol:scol + 1], s1=FMIN, imm2=1.0,
                accum_out=stats[:, ccol:ccol + 1])

        # ---- p*t diagonal extract
        diag = small.tile([P, MMW], F32, tag="diag")
        nc.vector.tensor_mul(diag[0:MMW, :], psum[0:MMW, 0:MMW],
                             ident[0:MMW, :])
        nc.vector.tensor_reduce(out=stats[0:MMW, C_PT:C_PT + 1],
                                in_=diag[0:MMW, :], axis=AX.X, op=Alu.add)
        nc.vector.tensor_copy(stats[:, C_SIG0:C_SIG0 + 1], sig[:, 0:1])
        nc.vector.tensor_copy(stats[:, C_SIGL:C_SIGL + 1], sig[:, F - 1:F])

        # ---- ACT Abs accumulators (after Ln in scalar-queue order)
        for k in range(NCH):
            w = FCH if k < NCH - 1 else FCH - 1
            nc.scalar.activation(out=dch[k][:, 0:w], in_=dch[k][:, 0:w],
                                 func=Act.Abs,
                                 accum_out=stats[:, C_SM0 + k:C_SM0 + k + 1])

        nc.sync.dma_start(out=stats_ext.ap(), in_=stats[:, :])

    nc.compile()
    return nc


_NC_CACHE = {}


def _get_nc():
    if "nc" not in _NC_CACHE:
        _NC_CACHE["nc"] = build_nc()
    return _NC_CACHE["nc"]


def _make_consts():
    import ml_dtypes
    ascB = np.broadcast_to(np.arange(1, NB + 1, dtype=np.float32), (P, NB))
    descB = np.broadcast_to(np.arange(NB, 0, -1, dtype=np.float32), (P, NB))
    korrB = np.broadcast_to(
        np.array([-1.0, -1.0, float(NB), float(NB)], np.float32), (P, 4))
    sgnB = np.broadcast_to(np.array([1.0, 1.0, -1.0, -1.0], np.float32), (P, 4))
    bigsB = np.broadcast_to(
        np.array([-BIGF, -BIGF, BIGF, BIGF], np.float32), (P, 4))
    offB = (float(NB) * (np.arange(P) % CHUNKS)).astype(np.float32).reshape(P, 1)
    i80 = (float(NB * CHUNKS) * np.arange(BPC)).astype(np.float32).reshape(BPC, 1)
    goff = np.broadcast_to(np.array(
        [0, 0, P * NB, P * NB,
         2 * P * NB, 2 * P * NB, 2 * P * NB, 2 * P * NB], np.float32),
        (BPC, 8))
    thr = np.zeros((64, 1), np.float32)
    thr[0:32] = 0.5
    rramp = np.zeros((64, PB), np.float32)
    j = np.arange(SUB, dtype=np.float32)
    rramp[0:16, 0:SUB] = SUB - j       # lo_t: desc
    rramp[16:32, 0:SUB] = j + 1        # hi_t: asc
    rramp[32:48, 0:SUB] = SUB - j      # lo_p: desc
    rramp[48:64, 0:SUB] = j + 1        # hi_p: asc
    ident = np.eye(P, MMW, dtype=np.float32)
    return {
        "ascB": np.ascontiguousarray(ascB),
        "descB": np.ascontiguousarray(descB),
        "korrB": np.ascontiguousarray(korrB),
        "sgnB": np.ascontiguousarray(sgnB),
        "bigsB": np.ascontiguousarray(bigsB),
        "offB": offB,
        "i80": i80,
        "goff": np.ascontiguousarray(goff),
        "thr": thr,
        "rramp": rramp.astype(np.float16),
        "ident": ident.astype(ml_dtypes.bfloat16),
    }


def _pad_blocks(arr, dtype):
    out = np.zeros((P * NB, PB), dtype=dtype)
    out.reshape(P, NB, PB)[:, :, 0:SUB] = arr.reshape(P, NB, SUB)
    return out


def make_in_maps(signals, predictions, targets):
    import ml_dtypes
    bf = ml_dtypes.bfloat16
    consts = _make_consts()
    s_all = np.ascontiguousarray(signals[:, 0, :]).astype(bf)
    p_all = np.ascontiguousarray(predictions[:, :, 0]).astype(bf)
    t_all = np.ascontiguousarray(targets[:, :, 0]).astype(bf)
    in_maps = []
    for i in range(N_CORES):
        sl = slice(i * BPC, (i + 1) * BPC)
        s_c = np.ascontiguousarray(s_all[sl].reshape(P, F))
        p_c = np.ascontiguousarray(p_all[sl].reshape(P, F))
        t_c = np.ascontiguousarray(t_all[sl].reshape(P, F))
        pad = np.concatenate([_pad_blocks(t_c, bf), _pad_blocks(p_c, bf),
                              _pad_blocks(s_c, bf)], axis=0)
        m = {"s": s_c, "p": p_c, "t": t_c, "tps_pad": pad}
        m.update(consts)
        in_maps.append(m)
    return in_maps


def host_combine(results):
    sp_sum = 0.0
    pt_sum = 0.0
    sm_sum = 0.0
    amp_sum = 0.0
    for res in results:
        stats = res["stats"].astype(np.float64)
        b16 = res["b16"].astype(np.float64)
        edge = res["edge"].astype(np.float64)
        rows = stats.reshape(BPC, CHUNKS, NSTAT)
        e4 = edge.reshape(4, BPC, 2)   # groups: lo_t, hi_t, lo_p, hi_p
        wmax_t = np.maximum(rows[:, :, C_WMAX_T].max(axis=1),
                            np.maximum(e4[0, :, 0], e4[1, :, 0]))
        wmin_t = np.minimum(-rows[:, :, C_WMIN_T].max(axis=1),
                            np.minimum(-e4[0, :, 1], -e4[1, :, 1]))
        wmax_p = np.maximum(rows[:, :, C_WMAX_P].max(axis=1),
                            np.maximum(e4[2, :, 0], e4[3, :, 0]))
        wmin_p = np.minimum(-rows[:, :, C_WMIN_P].max(axis=1),
                            np.minimum(-e4[2, :, 1], -e4[3, :, 1]))
        sp_sum += -rows[:, :, list(SP_COLS)].sum()
        pt_sum += rows[:, :, C_PT].sum()
        sm_sum += rows[:, :, C_SM0:C_SM0 + NCH].sum()
        sig0 = rows[:, :, C_SIG0]
        sigl = rows[:, :, C_SIGL]
        sm_sum += np.abs(sig0[:, 1:] - sigl[:, :-1]).sum()
        t_has = b16[:, 0] > -1e29
        p_has = b16[:, 1] > -1e29
        valid = t_has & p_has
        true_amp = (wmax_t - wmin_t).astype(np.float32)
        pred_amp = (wmax_p - wmin_p).astype(np.float32)
        d = np.abs(true_amp - pred_amp)
        per = np.where(true_amp > 1e-6, d / (true_amp + 1e-6), d)
        amp_sum += np.where(valid, per, 0.0).sum()
    bce = sp_sum / (B * L) - pt_sum / (B * L)
    amp = amp_sum / B
    smooth = sm_sum / (B * (L - 1))
    return np.float32(1.0 * bce + 0.5 * amp + 0.3 * smooth)


def kernel(signals, predictions, targets):
    nc = _get_nc()
    in_maps = make_in_maps(signals, predictions, targets)
    res = run_bass_kernel_spmd(nc, in_maps, core_ids=list(range(N_CORES)))
    return host_combine(res.results)


# revision 11
# speedup vs baseline: 1.1229x; 1.1229x over previous
"""AmpPerLoss distributed Trainium2 kernel (v3: engine-balanced pipeline).

Data-parallel over batch: 128 samples / 8 cores = 16 per core; each
sample's 100000-length row spans 8 SBUF partitions x 12500, so a core
shard is [128, 12500] (partition p = 8*sample + chunk).

Per core:
  - Loads ordered p -> t -> s in 2-block chunks; every consumer is
    chunk-pipelined behind its DMA.
  - BCE: sum softplus(p) = -sum ln(1 - sigmoid(p)) in ONE extra ACT pass
    (Ln with scale=-1, bias=1 reads the sigmoid tensor); sum p*t on the
    TensorEngine via 100 uniform 125-col accumulating matmuls + an
    identity-mask diagonal extract.
  - Window bounds: per-block maxes of t/p via 2x-packed halving folds
    (even inner dims keep DVE bf16 2x packing), tiny encode algebra,
    DRAM-bounce regroups, one fused gather (t+p+s pads concatenated in
    DRAM) and an exact in-block refine with per-row ramps/thresholds.
  - Windowed amplitude: interior from per-block min/max of s (masked
    block reduces), edges from gathered s blocks.
  - Smoothness: chunked shifted subtract of sigmoid (fp16, DVE 2x) +
    chunked ACT Abs accumulators; row-boundary pairs finished on host.

Host reduces the 8 cores' partial stats to the final scalar.
"""

import sys

if "/opt/trn_rl_repo" not in sys.path:
    sys.path.insert(0, "/opt/trn_rl_repo")

from contextlib import ExitStack

import numpy as np

import concourse.bass as bass
import concourse.bacc as bacc
import concourse.tile as tile
import concourse.mybir as mybir
from concourse.bass_utils import run_bass_kernel_spmd
from concourse import dve_ops

N_CORES = 8
B, L = 128, 100000
BPC = B // N_CORES          # samples per core
CHUNKS = 8                  # partitions per sample
P = BPC * CHUNKS            # 128 partitions
F = L // CHUNKS             # 12500 free elements per row
NB = 10                     # blocks per row
SUB = F // NB               # 1250
PB = 1280                   # padded block length in DRAM (256B aligned)
NCH = 2                     # load chunks per tensor (5 blocks each)
FCH = F // NCH              # 6250
H1, H2, H3, H4 = 626, 314, 158, 80   # even fold sizes
MMW = 125                   # matmul chunk width (100 uniform chunks)
NMM = F // MMW
NMM_PE = 70                 # chunks on PE; remainder via DVE TTR
BIGF = 1.0e30
FMIN = -3.0e38

F32 = mybir.dt.float32
BF16 = mybir.dt.bfloat16
FP16 = mybir.dt.float16
I16 = mybir.dt.int16
Alu = mybir.AluOpType
Act = mybir.ActivationFunctionType
AX = mybir.AxisListType

# stats column layout ([P, NSTAT])
C_WMAX_T, C_WMIN_T, C_WMAX_P, C_WMIN_P = 0, 1, 2, 3
C_SP, C_PT = 4, 5
C_SM0 = 6                   # NCH cols from here
C_SIG0, C_SIGL = 11, 12
SP_COLS = (4, 13)           # one per Ln chunk
C_PT2 = 15                  # DVE TTR partial of sum(p*t)
NSTAT = 16


def build_nc(n_cores=N_CORES):
    nc = bacc.Bacc("TRN2", target_bir_lowering=False, debug=False,
                   num_devices=n_cores)

    p_ext = nc.dram_tensor("p", [P, F], BF16, kind="ExternalInput")
    t_ext = nc.dram_tensor("t", [P, F], BF16, kind="ExternalInput")
    s_ext = nc.dram_tensor("s", [P, F], BF16, kind="ExternalInput")
    pad_ext = nc.dram_tensor("tps_pad", [3 * P * NB, PB], BF16,
                             kind="ExternalInput")
    ascB_ext = nc.dram_tensor("ascB", [P, NB], F32, kind="ExternalInput")
    descB_ext = nc.dram_tensor("descB", [P, NB], F32, kind="ExternalInput")
    korrB_ext = nc.dram_tensor("korrB", [P, 4], F32, kind="ExternalInput")
    sgnB_ext = nc.dram_tensor("sgnB", [P, 4], F32, kind="ExternalInput")
    bigsB_ext = nc.dram_tensor("bigsB", [P, 4], F32, kind="ExternalInput")
    offB_ext = nc.dram_tensor("offB", [P, 1], F32, kind="ExternalInput")
    i80_ext = nc.dram_tensor("i80", [BPC, 1], F32, kind="ExternalInput")
    goff_ext = nc.dram_tensor("goff", [BPC, 8], F32, kind="ExternalInput")
    thr_ext = nc.dram_tensor("thr", [64, 1], F32, kind="ExternalInput")
    rramp_ext = nc.dram_tensor("rramp", [64, PB], FP16, kind="ExternalInput")
    ident_ext = nc.dram_tensor("ident", [P, MMW], BF16, kind="ExternalInput")

    stats_ext = nc.dram_tensor("stats", [P, NSTAT], F32, kind="ExternalOutput")
    b16_ext = nc.dram_tensor("b16", [BPC, 4], F32, kind="ExternalOutput")
    edge_ext = nc.dram_tensor("edge", [64, 2], F32, kind="ExternalOutput")

    # DRAM bounce buffers (partition-crossing regroups)
    rv_b = nc.dram_tensor("rv_b", [P, 4], F32)
    pk_b = nc.dram_tensor("pk_b", [BPC, 4], F32)
    enc_b = nc.dram_tensor("enc_b", [64, 1], F32)
    sb_b = nc.dram_tensor("sb_b", [BPC, 8], F32)

    ctx = ExitStack()
    with tile.TileContext(nc) as tc, ctx:
        big = ctx.enter_context(tc.tile_pool(name="big", bufs=1))
        small = ctx.enter_context(tc.tile_pool(name="small", bufs=1))
        psum_pool = ctx.enter_context(
            tc.tile_pool(name="psum", bufs=1, space="PSUM"))

        p_sb = big.tile([P, F], BF16, tag="P")
        t_sb = big.tile([P, F], BF16, tag="T")
        s_sb = big.tile([P, F], BF16, tag="S")
        sig = big.tile([P, F], FP16, tag="SIG")

        # consts
        ascB = small.tile([P, NB], F32, tag="ascB")
        descB = small.tile([P, NB], F32, tag="descB")
        korrB = small.tile([P, 4], F32, tag="korrB")
        sgnB = small.tile([P, 4], F32, tag="sgnB")
        bigsB = small.tile([P, 4], F32, tag="bigsB")
        offB = small.tile([P, 1], F32, tag="offB")
        i80 = small.tile([BPC, 1], F32, tag="i80")
        goff = small.tile([BPC, 8], F32, tag="goff")
        thr = small.tile([64, 1], F32, tag="thr")
        rramp = small.tile([64, PB], FP16, tag="rramp")
        ident = small.tile([P, MMW], BF16, tag="ident")
        # ---- loads FIRST (p, then t, then s), consts behind them
        for k in range(NCH):
            sl = slice(k * FCH, (k + 1) * FCH)
            nc.sync.dma_start(out=p_sb[:, sl], in_=p_ext.ap()[:, sl])
        for k in range(NCH):
            sl = slice(k * FCH, (k + 1) * FCH)
            nc.sync.dma_start(out=t_sb[:, sl], in_=t_ext.ap()[:, sl])
        for k in range(NCH):
            sl = slice(k * FCH, (k + 1) * FCH)
            nc.sync.dma_start(out=s_sb[:, sl], in_=s_ext.ap()[:, sl])
        for sb, ext in ((ascB, ascB_ext), (descB, descB_ext),
                        (korrB, korrB_ext), (sgnB, sgnB_ext),
                        (bigsB, bigsB_ext), (offB, offB_ext),
                        (i80, i80_ext), (goff, goff_ext), (thr, thr_ext),
                        (rramp, rramp_ext), (ident, ident_ext)):
            nc.sync.dma_start(out=sb, in_=ext.ap())

        stats = small.tile([P, NSTAT], F32, tag="stats")
        nc.vector.memset(stats[:, :], 0.0)

        # ---- ACT: sigmoid chunks follow p
        for k in range(NCH):
            sl = slice(k * FCH, (k + 1) * FCH)
            nc.scalar.activation(out=sig[:, sl], in_=p_sb[:, sl],
                                 func=Act.Sigmoid)
        # ACT: sum ln(1 - sig) = -sum softplus(p), 2 chunks
        lndump = big.tile([P, F // 2], BF16, tag="LND")
        for k in range(2):
            sl = slice(k * (F // 2), (k + 1) * (F // 2))
            nc.scalar.activation(out=lndump[:, :], in_=sig[:, sl],
                                 func=Act.Ln, scale=-1.0, bias=1.0,
                                 accum_out=stats[:, SP_COLS[k]:SP_COLS[k] + 1])

        # ---- fold chains (DVE 2x): per-chunk fold1, then fold2-4 + reduce
        f1a = big.tile([P, NB, H1], BF16, tag="F1A")   # p, then s-max
        f1b = big.tile([P, NB, H1], BF16, tag="F1B")   # t, then s-min
        g2 = big.tile([P, NB, H2], BF16, tag="G2")
        g3 = big.tile([P, NB, H3], BF16, tag="G3")
        g4 = big.tile([P, NB, H4], BF16, tag="G4")
        bmax_p = small.tile([P, NB], F32, tag="bmax_p")
        bmax_t = small.tile([P, NB], F32, tag="bmax_t")
        bmax_s = small.tile([P, NB], F32, tag="bmax_s")
        bmin_s = small.tile([P, NB], F32, tag="bmin_s")

        NBH = NB // NCH         # blocks per load chunk
        def fold1(dst, src, k, op):
            v = src[:, k * FCH:(k + 1) * FCH].rearrange(
                "q (b f) -> q b f", b=NBH)
            nc.vector.tensor_tensor(out=dst[:, NBH * k:NBH * (k + 1), :],
                                    in0=v[:, :, 0:H1],
                                    in1=v[:, :, SUB - H1:SUB], op=op)

        def chain_tail(f1, out, op):
            nc.vector.tensor_tensor(out=g2[:, :, :], in0=f1[:, :, 0:H2],
                                    in1=f1[:, :, H1 - H2:H1], op=op)
            nc.vector.tensor_tensor(out=g3[:, :, :], in0=g2[:, :, 0:H3],
                                    in1=g2[:, :, H2 - H3:H2], op=op)
            nc.vector.tensor_tensor(out=g4[:, :, :], in0=g3[:, :, 0:H4],
                                    in1=g3[:, :, H3 - H4:H3], op=op)
            nc.vector.tensor_reduce(out=out[:, :], in_=g4[:, :, :],
                                    axis=AX.X, op=op)

        # smoothness sub chunks (fp16 2x); separate d tiles so ACT lag
        # never back-pressures DVE
        dch = []
        for k in range(NCH):
            d_k = big.tile([P, FCH], FP16, tag=f"D{k}", name=f"d{k}")
            dch.append(d_k)

        def sub_chunk(k):
            a = k * FCH
            w = FCH if k < NCH - 1 else FCH - 1
            nc.vector.tensor_sub(dch[k][:, 0:w], sig[:, a + 1:a + 1 + w],
                                 sig[:, a:a + w])

        # DVE: p-chain during p load
        for k in range(NCH):
            fold1(f1a, p_sb, k, Alu.max)
        chain_tail(f1a, bmax_p, Alu.max)
        # DVE: t-chain during t load, sub0 filling the arrival gap
        fold1(f1b, t_sb, 0, Alu.max)
        sub_chunk(0)
        fold1(f1b, t_sb, 1, Alu.max)
        chain_tail(f1b, bmax_t, Alu.max)

        # ---- matmuls: p^T @ t -- PE takes NMM_PE chunks, DVE the rest
        psum = psum_pool.tile([P, MMW], F32)
        for c in range(NMM_PE):
            nc.tensor.matmul(out=psum[0:MMW, 0:MMW],
                             lhsT=p_sb[:, c * MMW:(c + 1) * MMW],
                             rhs=t_sb[:, c * MMW:(c + 1) * MMW],
                             start=(c == 0), stop=(c == NMM_PE - 1))

        # ---- block-level bound encode (tiny)
        anyt = small.tile([P, NB], F32, tag="anyt")
        anyp = small.tile([P, NB], F32, tag="anyp")
        nc.vector.tensor_scalar(out=anyt[:, :], in0=bmax_t[:, :],
                                scalar1=0.5, scalar2=None, op0=Alu.is_gt)
        nc.vector.tensor_scalar(out=anyp[:, :], in0=bmax_p[:, :],
                                scalar1=0.0, scalar2=None, op0=Alu.is_gt)
        encB = small.tile([P, 4], F32, tag="encB")  # [hi_t, hi_p, lo_t, lo_p]
        ze = small.tile([P, NB], F32, tag="ze")
        for i, (src, rmp) in enumerate(((anyt, ascB), (anyp, ascB),
                                        (anyt, descB), (anyp, descB))):
            nc.vector.tensor_mul(ze[:, :], src[:, :], rmp[:, :])
            nc.vector.tensor_reduce(out=encB[:, i:i + 1], in_=ze[:, :],
                                    axis=AX.X, op=Alu.max)
        # hi cols: g = enc-1 + 10c (else -BIG); lo: g = 10-enc + 10c (else BIG)
        cm = small.tile([P, 4], F32, tag="cm")
        dm = small.tile([P, 4], F32, tag="dm")
        a1 = small.tile([P, 4], F32, tag="a1")
        t1 = small.tile([P, 4], F32, tag="t1")
        t2 = small.tile([P, 4], F32, tag="t2")
        rowvals = small.tile([P, 4], F32, tag="rowvals")
        nc.vector.tensor_scalar(out=cm[:, :], in0=encB[:, :], scalar1=0.0,
                                scalar2=None, op0=Alu.is_gt)
        nc.vector.tensor_scalar(out=dm[:, :], in0=encB[:, :], scalar1=0.0,
                                scalar2=None, op0=Alu.is_le)
        nc.vector.tensor_mul(a1[:, :], encB[:, :], sgnB[:, :])
        nc.vector.tensor_add(a1[:, :], a1[:, :], korrB[:, :])
        nc.vector.tensor_scalar(out=a1[:, :], in0=a1[:, :],
                                scalar1=offB[:, 0:1], scalar2=None,
                                op0=Alu.add)
        nc.vector.tensor_mul(t1[:, :], cm[:, :], a1[:, :])
        nc.vector.tensor_mul(t2[:, :], dm[:, :], bigsB[:, :])
        nc.vector.tensor_add(rowvals[:, :], t1[:, :], t2[:, :])

        # ---- per-sample combine: bounce [P,4] -> [16,4,8], reduce over 8
        comb = small.tile([BPC, CHUNKS, 4], F32, tag="comb")
        nc.sync.dma_start(out=rv_b.ap(), in_=rowvals[:, :])
        rap = rv_b.ap()
        # comb[i, c, k] = rv_b[8i + c, k]
        nc.sync.dma_start(out=comb[:, :, :], in_=bass.AP(
            tensor=rap.tensor, offset=rap.offset,
            ap=[[4 * CHUNKS, BPC], [4, CHUNKS], [1, 4]]))
        combv = comb[:, :, :].rearrange("b c k -> b k c")
        pack = small.tile([BPC, 4], F32, tag="pack")
        nc.vector.tensor_reduce(out=pack[:, 0:2], in_=combv[:, 0:2, :],
                                axis=AX.X, op=Alu.max)
        nc.vector.tensor_reduce(out=pack[:, 2:4], in_=combv[:, 2:4, :],
                                axis=AX.X, op=Alu.min)
        nc.sync.dma_start(out=b16_ext.ap(), in_=pack[:, :])

        # eq per mask: single-block window (lo_g == hi_g)
        eq2 = small.tile([BPC, 2], F32, tag="eq2")
        nc.vector.tensor_tensor(out=eq2[:, :], in0=pack[:, 2:4],
                                in1=pack[:, 0:2], op=Alu.is_equal)

        # ---- gather indices: idx8 = clamp(g,0,79) + 80*i + goff
        gcl = small.tile([BPC, 4], F32, tag="gcl")
        for dst_c, src_c in ((0, 2), (1, 0), (2, 3), (3, 1)):
            nc.vector.tensor_copy(gcl[:, dst_c:dst_c + 1],
                                  pack[:, src_c:src_c + 1])
        nc.vector.tensor_scalar(out=gcl[:, :], in0=gcl[:, :],
                                scalar1=0.0, scalar2=79.0,
                                op0=Alu.max, op1=Alu.min)
        nc.vector.tensor_scalar(out=gcl[:, :], in0=gcl[:, :],
                                scalar1=i80[:, 0:1], scalar2=None,
                                op0=Alu.add)
        idx8 = small.tile([BPC, 8], F32, tag="idx8")
        nc.vector.tensor_copy(idx8[:, 0:4], gcl[:, :])
        nc.vector.tensor_copy(idx8[:, 4:8], gcl[:, :])
        nc.vector.tensor_add(idx8[:, :], idx8[:, :], goff[:, :])
        idx_i = small.tile([BPC, 8], I16, tag="idx_i")
        nc.vector.tensor_copy(idx_i[:, :], idx8[:, :])

        # ---- broadcast per-sample bounds to rows (interior masks)
        rb = small.tile([P, 4], F32, tag="rb")
        nc.sync.dma_start(out=pk_b.ap(), in_=pack[:, :])
        pap = pk_b.ap()
        # rb[8i + c, :] = pk_b[i, :]
        nc.sync.dma_start(out=rb[:, :], in_=bass.AP(
            tensor=pap.tensor, offset=pap.offset,
            ap=[[4, BPC], [0, CHUNKS], [1, 4]]))

        # ---- gathers (fused pad tensor; manual DMA semaphores)
        gref = small.tile([P, 1, PB], BF16, tag="gref")
        sgat = small.tile([P, 1, PB], BF16, tag="sgat")
        from concourse.bass import _add_dep_helper
        gsem_r = nc.alloc_semaphore("gsem_r")
        gsem_s = nc.alloc_semaphore("gsem_s")
        nc.gpsimd.dma_gather(
            out_ap=gref[:, :, :], in_ap=pad_ext.ap(),
            idxs_ap=idx_i[:, 0:4], num_idxs=64,
            num_idxs_reg=64, elem_size=PB,
            prepare_only=True, sem=gsem_r)
        nc.gpsimd.dma_gather(
            out_ap=sgat[:, :, :], in_ap=pad_ext.ap(),
            idxs_ap=idx_i[:, 4:8], num_idxs=64,
            num_idxs_reg=64, elem_size=PB,
            prepare_only=True, sem=gsem_s)
        trig = nc.gpsimd.trigger_dma(count=None)
        gw = {}
        for key, sem in (("r", gsem_r), ("s", gsem_s)):
            w = nc.gpsimd.wait_ge(sem, 16)
            _add_dep_helper(w.ins, trig.ins, sync=False,
                            reason="gather wait after trigger")
            gw[key] = w

        def dep_on_gather(inst, key):
            _add_dep_helper(inst.ins, gw[key].ins, sync=True,
                            reason=f"reader waits {key}-gather")

        # ---- s fold chains (interleaved max/min) + refine parked mid-way
        sub_chunk(1)
        ttr_dump = big.tile([P, F - NMM_PE * MMW], BF16, tag="TTRD")
        nc.vector._custom_dve(
            dve_ops.TENSOR_TENSOR_REDUCE,
            out=ttr_dump[:, :], in0=p_sb[:, NMM_PE * MMW:F],
            in1=t_sb[:, NMM_PE * MMW:F], s0=0.0, s1=1.0,
            accum_out=stats[:, C_PT2:C_PT2 + 1])
        fold1(f1a, s_sb, 0, Alu.max)
        fold1(f1b, s_sb, 0, Alu.min)

        # refine exact in-block positions (rows: lo_t, hi_t, lo_p, hi_p);
        # these park in DVE wait slots until the gather semaphore fires
        refm = small.tile([64, PB], FP16, tag="refm")
        refe = small.tile([64, PB], FP16, tag="refe")
        enc = small.tile([64, 1], F32, tag="enc")
        r_ = nc.vector.tensor_scalar(out=refm[:, :], in0=gref[0:64, 0, :],
                                     scalar1=thr[:, 0:1], scalar2=None,
                                     op0=Alu.is_gt)
        dep_on_gather(r_, "r")
        nc.vector.tensor_mul(refe[:, :], refm[:, :], rramp[:, :])
        nc.vector.tensor_reduce(out=enc[:, :], in_=refe[:, :],
                                axis=AX.X, op=Alu.max)
        # regroup enc -> encs16[i, g]
        encs16 = small.tile([BPC, 4], F32, tag="encs16")
        nc.sync.dma_start(out=enc_b.ap(), in_=enc[:, :])
        eap = enc_b.ap()
        nc.sync.dma_start(out=encs16[:, :], in_=bass.AP(
            tensor=eap.tensor, offset=eap.offset,
            ap=[[1, BPC], [BPC, 4]]))

        # sgneg for the min-edge reduce (dep on s-gather)
        sgneg = small.tile([64, PB], BF16, tag="sgneg")
        r_ = nc.vector.tensor_scalar(out=sgneg[:, :], in0=sgat[0:64, 0, :],
                                     scalar1=-1.0, scalar2=None, op0=Alu.mult)
        dep_on_gather(r_, "s")

        fold1(f1a, s_sb, 1, Alu.max)
        fold1(f1b, s_sb, 1, Alu.min)

        # ---- per-row [start, end) for gathered s blocks
        # encs16 cols: [enc_lo_t, enc_hi_t, enc_lo_p, enc_hi_p]
        sb8 = small.tile([BPC, 8], F32, tag="sb8")
        tmp2 = small.tile([BPC, 2], F32, tag="tmp2")
        for c, ec in ((0, 0), (4, 2)):   # st_lo = 1250 - enc_lo
            nc.vector.tensor_scalar(
                out=sb8[:, c:c + 1], in0=encs16[:, ec:ec + 1],
                scalar1=-1.0, scalar2=float(SUB), op0=Alu.mult, op1=Alu.add)
        for c, ec in ((0, 1), (1, 3)):   # en_lo = 1250 + eq*(enc_hi - 1250)
            nc.vector.tensor_scalar(
                out=tmp2[:, c:c + 1], in0=encs16[:, ec:ec + 1],
                scalar1=-float(SUB), scalar2=None, op0=Alu.add)
        nc.vector.tensor_mul(tmp2[:, :], tmp2[:, :], eq2[:, :])
        for c in (0, 1):
            nc.vector.tensor_scalar(
                out=sb8[:, 4 * c + 1:4 * c + 2], in0=tmp2[:, c:c + 1],
                scalar1=float(SUB), scalar2=None, op0=Alu.add)
        for c in (0, 1):                 # st_hi = eq * st_lo
            nc.vector.tensor_mul(sb8[:, 4 * c + 2:4 * c + 3],
                                 eq2[:, c:c + 1], sb8[:, 4 * c:4 * c + 1])
        for c, ec in ((0, 1), (1, 3)):   # en_hi = enc_hi
            nc.vector.tensor_copy(sb8[:, 4 * c + 3:4 * c + 4],
                                  encs16[:, ec:ec + 1])
        sbnd = small.tile([64, 2], F32, tag="sbnd")
        nc.sync.dma_start(out=sb_b.ap(), in_=sb8[:, :])
        sap = sb_b.ap()
        nc.sync.dma_start(out=sbnd[:, :], in_=bass.AP(
            tensor=sap.tensor, offset=sap.offset,
            ap=[[2, 4], [8, BPC], [1, 2]]))

        # ---- edge extremes from gathered s blocks
        edge = small.tile([64, 2], F32, tag="edge")
        edump = small.tile([64, PB], BF16, tag="edump")
        r_ = nc.vector._custom_dve(
            dve_ops.TENSOR_MASK_REDUCE,
            out=edump[:, :], in0=sgat[0:64, 0, :], in1=sbnd[:, 1:2],
            s0=sbnd[:, 0:1], s1=FMIN, imm2=1.0, accum_out=edge[:, 0:1])
        dep_on_gather(r_, "s")
        nc.vector._custom_dve(
            dve_ops.TENSOR_MASK_REDUCE,
            out=edump[:, :], in0=sgneg[:, :], in1=sbnd[:, 1:2],
            s0=sbnd[:, 0:1], s1=FMIN, imm2=1.0, accum_out=edge[:, 1:2])
        nc.sync.dma_start(out=edge_ext.ap(), in_=edge[:, :])

        # ---- s chain tails
        chain_tail(f1a, bmax_s, Alu.max)
        chain_tail(f1b, bmin_s, Alu.min)

        # ---- interior extremes from block stats (masked block reduces)
        ibs = small.tile([P, 2], F32, tag="ibs")
        ibe = small.tile([P, 2], F32, tag="ibe")
        nc.vector.tensor_scalar(out=ibs[:, :], in0=rb[:, 2:4],
                                scalar1=offB[:, 0:1], scalar2=1.0,
                                op0=Alu.subtract, op1=Alu.add)
        nc.vector.tensor_scalar(out=ibe[:, :], in0=rb[:, 0:2],
                                scalar1=offB[:, 0:1], scalar2=None,
                                op0=Alu.subtract)
        nc.vector.tensor_tensor(out=ibs[:, :], in0=ibs[:, :], in1=ibe[:, :],
                                op=Alu.min)
        negb = small.tile([P, NB], F32, tag="negb")
        nc.vector.tensor_scalar(out=negb[:, :], in0=bmin_s[:, :],
                                scalar1=-1.0, scalar2=None, op0=Alu.mult)
        bdump = small.tile([P, NB], F32, tag="bdump")
        for (data, scol, ccol) in ((bmax_s, 0, C_WMAX_T), (negb, 0, C_WMIN_T),
                                   (bmax_s, 1, C_WMAX_P), (negb, 1, C_WMIN_P)):
            nc.vector._custom_dve(
                dve_ops.TENSOR_MASK_REDUCE,
                out=bdump[:, :], in0=data[:, :], in1=ibe[:, scol:scol + 1],
                s0=ibs[:, scol:scol + 1], s1=FMIN, imm2=1.0,
                accum_out=stats[:, ccol:ccol + 1])

        # ---- p*t diagonal extract
        diag = small.tile([P, MMW], F32, tag="diag")
        nc.vector.tensor_mul(diag[0:MMW, :], psum[0:MMW, 0:MMW],
                             ident[0:MMW, :])
        nc.vector.tensor_reduce(out=stats[0:MMW, C_PT:C_PT + 1],
                                in_=diag[0:MMW, :], axis=AX.X, op=Alu.add)
        nc.vector.tensor_copy(stats[:, C_SIG0:C_SIG0 + 1], sig[:, 0:1])
        nc.vector.tensor_copy(stats[:, C_SIGL:C_SIGL + 1], sig[:, F - 1:F])

        # ---- ACT Abs accumulators (after Ln in scalar-queue order)
        for k in range(NCH):
            w = FCH if k < NCH - 1 else FCH - 1
            nc.scalar.activation(out=dch[k][:, 0:w], in_=dch[k][:, 0:w],
                                 func=Act.Abs,
                                 accum_out=stats[:, C_SM0 + k:C_SM0 + k + 1])

        nc.sync.dma_start(out=stats_ext.ap(), in_=stats[:, :])

    nc.compile()
    return nc


_NC_CACHE = {}


def _get_nc():
    if "nc" not in _NC_CACHE:
        _NC_CACHE["nc"] = build_nc()
    return _NC_CACHE["nc"]


def _make_consts():
    import ml_dtypes
    ascB = np.broadcast_to(np.arange(1, NB + 1, dtype=np.float32), (P, NB))
    descB = np.broadcast_to(np.arange(NB, 0, -1, dtype=np.float32), (P, NB))
    korrB = np.broadcast_to(
        np.array([-1.0, -1.0, float(NB), float(NB)], np.float32), (P, 4))
    sgnB = np.broadcast_to(np.array([1.0, 1.0, -1.0, -1.0], np.float32), (P, 4))
    bigsB = np.broadcast_to(
        np.array([-BIGF, -BIGF, BIGF, BIGF], np.float32), (P, 4))
    offB = (float(NB) * (np.arange(P) % CHUNKS)).astype(np.float32).reshape(P, 1)
    i80 = (float(NB * CHUNKS) * np.arange(BPC)).astype(np.float32).reshape(BPC, 1)
    goff = np.broadcast_to(np.array(
        [0, 0, P * NB, P * NB,
         2 * P * NB, 2 * P * NB, 2 * P * NB, 2 * P * NB], np.float32),
        (BPC, 8))
    thr = np.zeros((64, 1), np.float32)
    thr[0:32] = 0.5
    rramp = np.zeros((64, PB), np.float32)
    j = np.arange(SUB, dtype=np.float32)
    rramp[0:16, 0:SUB] = SUB - j       # lo_t: desc
    rramp[16:32, 0:SUB] = j + 1        # hi_t: asc
    rramp[32:48, 0:SUB] = SUB - j      # lo_p: desc
    rramp[48:64, 0:SUB] = j + 1        # hi_p: asc
    ident = np.eye(P, MMW, dtype=np.float32)
    return {
        "ascB": np.ascontiguousarray(ascB),
        "descB": np.ascontiguousarray(descB),
        "korrB": np.ascontiguousarray(korrB),
        "sgnB": np.ascontiguousarray(sgnB),
        "bigsB": np.ascontiguousarray(bigsB),
        "offB": offB,
        "i80": i80,
        "goff": np.ascontiguousarray(goff),
        "thr": thr,
        "rramp": rramp.astype(np.float16),
        "ident": ident.astype(ml_dtypes.bfloat16),
    }


def _pad_blocks(arr, dtype):
    out = np.zeros((P * NB, PB), dtype=dtype)
    out.reshape(P, NB, PB)[:, :, 0:SUB] = arr.reshape(P, NB, SUB)
    return out


def make_in_maps(signals, predictions, targets):
    import ml_dtypes
    bf = ml_dtypes.bfloat16
    consts = _make_consts()
    s_all = np.ascontiguousarray(signals[:, 0, :]).astype(bf)
    p_all = np.ascontiguousarray(predictions[:, :, 0]).astype(bf)
    t_all = np.ascontiguousarray(targets[:, :, 0]).astype(bf)
    in_maps = []
    for i in range(N_CORES):
        sl = slice(i * BPC, (i + 1) * BPC)
        s_c = np.ascontiguousarray(s_all[sl].reshape(P, F))
        p_c = np.ascontiguousarray(p_all[sl].reshape(P, F))
        t_c = np.ascontiguousarray(t_all[sl].reshape(P, F))
        pad = np.concatenate([_pad_blocks(t_c, bf), _pad_blocks(p_c, bf),
                              _pad_blocks(s_c, bf)], axis=0)
        m = {"s": s_c, "p": p_c, "t": t_c, "tps_pad": pad}
        m.update(consts)
        in_maps.append(m)
    return in_maps


def host_combine(results):
    sp_sum = 0.0
    pt_sum = 0.0
    sm_sum = 0.0
    amp_sum = 0.0
    for res in results:
        stats = res["stats"].astype(np.float64)
        b16 = res["b16"].astype(np.float64)
        edge = res["edge"].astype(np.float64)
        rows = stats.reshape(BPC, CHUNKS, NSTAT)
        e4 = edge.reshape(4, BPC, 2)   # groups: lo_t, hi_t, lo_p, hi_p
        wmax_t = np.maximum(rows[:, :, C_WMAX_T].max(axis=1),
                            np.maximum(e4[0, :, 0], e4[1, :, 0]))
        wmin_t = np.minimum(-rows[:, :, C_WMIN_T].max(axis=1),
                            np.minimum(-e4[0, :, 1], -e4[1, :, 1]))
        wmax_p = np.maximum(rows[:, :, C_WMAX_P].max(axis=1),
                            np.maximum(e4[2, :, 0], e4[3, :, 0]))
        wmin_p = np.minimum(-rows[:, :, C_WMIN_P].max(axis=1),
                            np.minimum(-e4[2, :, 1], -e4[3, :, 1]))
        sp_sum += -rows[:, :, list(SP_COLS)].sum()
        pt_sum += rows[:, :, C_PT2].sum()
        pt_sum += rows[:, :, C_PT].sum()
        sm_sum += rows[:, :, C_SM0:C_SM0 + NCH].sum()
        sig0 = rows[:, :, C_SIG0]
        sigl = rows[:, :, C_SIGL]
        sm_sum += np.abs(sig0[:, 1:] - sigl[:, :-1]).sum()
        t_has = b16[:, 0] > -1e29
        p_has = b16[:, 1] > -1e29
        valid = t_has & p_has
        true_amp = (wmax_t - wmin_t).astype(np.float32)
        pred_amp = (wmax_p - wmin_p).astype(np.float32)
        d = np.abs(true_amp - pred_amp)
        per = np.where(true_amp > 1e-6, d / (true_amp + 1e-6), d)
        amp_sum += np.where(valid, per, 0.0).sum()
    bce = sp_sum / (B * L) - pt_sum / (B * L)
    amp = amp_sum / B
    smooth = sm_sum / (B * (L - 1))
    return np.float32(1.0 * bce + 0.5 * amp + 0.3 * smooth)


def kernel(signals, predictions, targets):
    nc = _get_nc()
    in_maps = make_in_maps(signals, predictions, targets)
    res = run_bass_kernel_spmd(nc, in_maps, core_ids=list(range(N_CORES)))
    return host_combine(res.results)
